# revision 1
# baseline (speedup 1.0000x reference)
"""Multi-head self-attention Trainium2 kernel (8 NeuronCores).

Problem: B=4, S=2048, K=128 head_dim, H=8 heads, fp32.
Sharding: batch*head-group parallel — core i computes batch b=i//2 and the
4 heads hg=i%2 (heads hg*4..hg*4+3), producing a partial output
y_part[b] = sum_{h in group} softmax(q_h k_h^T) v_h @ Wo_h.  Host adds the
two partials per batch plus bias.

The host passes x already transposed (xT, [128c, S]) and all inputs
pre-rounded to tf32 bit patterns; the kernel returns yT ([128c, S]) which
the host transposes back.  This removes all on-device transposes.

Per-core structure (matmuls in float32r = tf32, fp32 accumulate):
  v_t  = x_t @ Wv (all 4 heads at once)    [128s, 512]  per s-chunk t
  qT_h, kT_h = weight-stationary matmuls   [d, S]
  per (head, q-block of 512) x (k-chunk PAIR of 2x128):
    scoresT pair = 2 matmuls               [128k, 2x512q] in one PSUM tile
    one exp on ScalarE PSUM->SBUF [128,1024] (no max subtraction)
    outT  += v_chunk-stationary @ exp_half (PSUM accumulate, 16 k-chunks)
    denom: 8 tiny matmuls (exp slice as STATIONARY, ones[128,2] moving,
           out free 2 ~ 3ns) accumulate sum_k exp into dall[128q, 8] -
           one psum group per block, drained inline at the last pair
  per block: mask-mul (ident x d0c, stride-0 APs) -> [1,512] row fold ->
           reciprocal -> ONE stride-0 broadcast DMA (SP queue; a PE rank-1
           matmul at the kernel tail instead of the DMA round trip)
  outTn = outT * bcs                       (DVE mult)
  yT   += Wo_h-stationary over outTn       (SBUF fp32 accumulate over heads)

All 16 (head, q-block) blocks run as ONE software-pipelined stream: the
attn@v/denominator consumer lags the scores/exp producer by two k-pairs,
crossing block boundaries without refill bubbles.  Block epilogues
(normalize, output projection) and next-head q/k projections are closures
drip-fed into the stream, epilogues first (they release PSUM slots).
"""

import os

import numpy as np

WARM = int(os.environ.get("KERNEL_WARM", "3"))
LAG = int(os.environ.get("KERNEL_LAG", "6"))

P = 128
S = 2048
NH = 4  # heads per core
SC = S // P  # 16 k-chunks
NP = SC // 2  # 8 k-pairs
NQ = S // 512  # 4 q-blocks per head
N_CORES = 8

_CACHE = {}
LAST_RESULTS = None


def _tf32_round(a):
    """Round fp32 array to tf32 (10-bit mantissa) bit patterns, RNE."""
    bits = np.ascontiguousarray(a, dtype=np.float32).view(np.uint32)
    rounded = bits + np.uint32(0x0FFF) + ((bits >> np.uint32(13)) & np.uint32(1))
    rounded &= np.uint32(0xFFFFE000)
    return rounded.view(np.float32)


def _build():
    from contextlib import ExitStack

    import concourse.bass as bass
    import concourse.tile as tile
    from concourse import bacc, mybir

    f32 = mybir.dt.float32
    f32r = mybir.dt.float32r
    Exp = mybir.ActivationFunctionType.Exp

    nc = bacc.Bacc("TRN2", target_bir_lowering=False, debug=False,
                   num_devices=N_CORES)
    xt = nc.dram_tensor("xt", [P, S], f32r, kind="ExternalInput").ap()
    wq = nc.dram_tensor("wq", [P, NH * P], f32r, kind="ExternalInput").ap()
    wk = nc.dram_tensor("wk", [P, NH * P], f32r, kind="ExternalInput").ap()
    wv = nc.dram_tensor("wv", [P, NH * P], f32r, kind="ExternalInput").ap()
    wo = nc.dram_tensor("wo", [P, NH * P], f32r, kind="ExternalInput").ap()
    y = nc.dram_tensor("y", [P, S], f32, kind="ExternalOutput").ap()

    with tile.TileContext(nc) as tc, ExitStack() as ctx:
        consts = ctx.enter_context(tc.tile_pool(name="consts", bufs=1))
        bigs = ctx.enter_context(tc.tile_pool(name="bigs", bufs=1))
        qkp = ctx.enter_context(tc.tile_pool(name="qkp", bufs=2))
        expp = ctx.enter_context(tc.tile_pool(name="expp", bufs=8))
        outp = ctx.enter_context(tc.tile_pool(name="outp", bufs=2))
        small = ctx.enter_context(tc.tile_pool(name="small", bufs=4))
        # PSUM banks: stage 2x[128,1024]=4 + outT/prefetch 2x[128,512]=2
        #             + den/epilogue 2x[128,512]=2  -> 8 banks
        psStage = ctx.enter_context(tc.tile_pool(name="psStage", bufs=2, space="PSUM"))
        psOut = ctx.enter_context(tc.tile_pool(name="psOut", bufs=2, space="PSUM"))
        psDen = ctx.enter_context(tc.tile_pool(name="psDen", bufs=1, space="PSUM"))
        psDall = ctx.enter_context(tc.tile_pool(name="psDall", bufs=1, space="PSUM"))

        # round-robin over the copy engines for psum->sbuf drains
        cp_rr = [0]

        def ew_copy(dst_ap, src_ap):
            # psum->sbuf drains must run on DVE: GPSIMD cannot access PSUM
            nc.vector.tensor_copy(dst_ap, src_ap)

        # --- DMAs: xT first (needed earliest), two HWDGE queues ---
        xT = bigs.tile([P, S], f32r)
        wq_r = consts.tile([P, NH * P], f32r)
        wk_r = consts.tile([P, NH * P], f32r)
        wv_r = consts.tile([P, NH * P], f32r)
        wo_r = consts.tile([P, NH * P], f32r)
        # ALL loads ride the SP queue: a dma_start costs ~1.3us of the
        # issuing engine's SEQUENCER, and anything on nc.scalar's sequencer
        # delays every exp behind it.  Order = first-needed-first.
        nc.sync.dma_start(xT[:, 0:512], xt[:, 0:512])
        nc.sync.dma_start(wq_r[:, 0:P], wq[:, 0:P])
        nc.sync.dma_start(wk_r[:, 0:P], wk[:, 0:P])
        nc.sync.dma_start(wv_r[:], wv[:])
        nc.sync.dma_start(xT[:, 512:1024], xt[:, 512:1024])
        nc.sync.dma_start(xT[:, 1024:1536], xt[:, 1024:1536])
        nc.sync.dma_start(xT[:, 1536:2048], xt[:, 1536:2048])
        nc.sync.dma_start(wk_r[:, P:], wk[:, P:])
        nc.sync.dma_start(wq_r[:, P:], wq[:, P:])
        nc.sync.dma_start(wo_r[:], wo[:])

        # ONE psum bank shared by the denominator folds (cols 0:8, one
        # accumulation group per block, drained inline at the last pair)
        # and the PE warm-up target (row 0, temporally disjoint)
        dall = psDall.tile([P, 512], f32, name="dall", tag="dall")
        # memset straight into the f32r tiles (1.0's bits are tf32-clean);
        # the warm chain goes first so PE pre-heating starts ~1.5us earlier
        ones_col = consts.tile([P, 1], f32r)
        nc.vector.memset(ones_col.bitcast(f32)[:], 1.0)
        warm_r = consts.tile([P, 512], f32r)
        nc.vector.memset(warm_r.bitcast(f32)[:], 1.0)
        # pre-heat the PE during the DMA dead zone: dummy matmuls release
        # the HAM clock gate (1.2 -> 2.4 GHz) before real work lands
        for i in range(WARM):
            nc.tensor.matmul(dall[0:1, :], ones_col[:], warm_r[:],
                             start=True, stop=True)
        # [128,2] ones: moving operand of the tiny denominator folds (the
        # ISA rejects 1-column moving operands, so fold into 2 dup columns)
        ones_c2 = consts.tile([P, 2], f32r)
        nc.gpsimd.memset(ones_c2.bitcast(f32)[:], 1.0)
        ones_row_f = consts.tile([1, P], f32)
        nc.gpsimd.memset(ones_row_f[:], 1.0)
        # [128,128] identity for PE transposes of the per-block denominators
        warm_f = warm_r.bitcast(f32)
        ident = consts.tile([P, P], f32)
        nc.gpsimd.affine_select(ident[:], warm_f[:, 0:P], [[1, P]],
                                mybir.AluOpType.is_equal, 0.0,
                                base=0, channel_multiplier=-1)

        v_sb = bigs.tile([P, SC * 512], f32r)
        yT = bigs.tile([P, S], f32)

        heads_qk = {0: (qkp.tile([P, S], f32r, name="qT0", tag="qT"),
                        qkp.tile([P, S], f32r, name="kT0", tag="kT"))}

        def qk_step(h, w_r, dst, qc, tag, pool, eng=None, via_dma=False):
            ps = pool.tile([P, 512], f32, name=f"qk{h}{qc}", tag=tag)
            nc.tensor.matmul(ps[:], w_r[:, h * P:(h + 1) * P],
                             xT[:, qc * 512:(qc + 1) * 512],
                             start=True, stop=True)
            dst_sl = dst[:, qc * 512:(qc + 1) * 512]
            if via_dma:
                ew_copy(dst_sl, ps[:])
            else:
                (eng or nc.vector.tensor_copy)(dst_sl, ps[:])

        # --- progressive startup per group of 4 s-chunks ---
        qT0, kT0 = heads_qk[0]

        def qk_pair(w_r, dst, gp):
            ps = pool_tile = psStage.tile([P, 1024], f32, name=f"qkp{gp}",
                                          tag="stage")
            for j in range(2):
                qc = gp * 2 + j
                nc.tensor.matmul(ps[:, j * 512:(j + 1) * 512],
                                 w_r[:, 0:P], xT[:, qc * 512:(qc + 1) * 512],
                                 start=True, stop=True)
            nc.vector.tensor_copy(dst[:, gp * 1024:(gp + 1) * 1024], ps[:])

        def v_pair(tp):
            psvh = psStage.tile([P, 1024], f32, name=f"psv{tp}", tag="stage")
            for j in range(2):
                t = tp * 2 + j
                nc.tensor.matmul(psvh[:, j * 512:(j + 1) * 512],
                                 xT[:, t * P:(t + 1) * P], wv_r[:],
                                 start=True, stop=True)
            nc.vector.tensor_copy(v_sb[:, tp * 1024:(tp + 1) * 1024],
                                  psvh[:])

        # critical-path first: ONLY the two projections scores(0,0) needs;
        # everything else (v, later k/q chunks) drips inside the stream so
        # the first exp isn't queued behind background PE/DVE work.  The k
        # copy rides the (idle) Act engine so both copies run in parallel.
        qk_step(0, wq_r, qT0, 0, "stage", psStage, eng=nc.scalar.copy)
        qk_step(0, wk_r, kT0, 0, "stage", psStage)

        def v_step(t, pool, tag):
            ps = pool.tile([P, 512], f32, name=f"vs{t}", tag=tag)
            nc.tensor.matmul(ps[:], xT[:, t * P:(t + 1) * P], wv_r[:],
                             start=True, stop=True)
            nc.vector.tensor_copy(v_sb[:, t * 512:(t + 1) * 512], ps[:])

        def qk_bg(w_r, dst, qc, po):
            return lambda: qk_step(0, w_r, dst, qc,
                                   "po" if po else "den",
                                   psOut if po else psDen)

        # drip order tuned to each item's deadline (2 pops per j-iter):
        # k0 chunks feed scores pairs 2/4/6, q0c1 feeds block 1, v chunk t
        # feeds the lagged consumer of pair t//2
        startup_bg = []
        vs = [(lambda t=t, po=po: v_step(t, psOut if po else psDen,
                                         "po" if po else "den"))
              for t, po in ((t, t % 2 == 0) for t in range(SC))]
        startup_bg = [
            lambda: qk_step(0, wk_r, kT0, 1, "stage", psStage),
            vs[0], vs[1], vs[2],
            qk_bg(wk_r, kT0, 2, True), vs[3], vs[4],
            qk_bg(wk_r, kT0, 3, False), vs[5],
            qk_bg(wq_r, qT0, 1, True), vs[6], vs[7], vs[8], vs[9],
            vs[10], vs[11], vs[12], vs[13], vs[14], vs[15],
            qk_bg(wq_r, qT0, 2, False), qk_bg(wq_r, qT0, 3, True),
        ]

        # --- main stream over 16 blocks, iterating k-pairs ---
        blocks = [(h, qcb) for h in range(NH) for qcb in range(NQ)]
        NB = len(blocks)
        bstate = {}
        bg_epi = []
        bg_pre = []

        def make_prefetch(h):
            nqT = qkp.tile([P, S], f32r, name=f"qT{h}", tag="qT")
            nkT = qkp.tile([P, S], f32r, name=f"kT{h}", tag="kT")
            heads_qk[h] = (nqT, nkT)
            steps = []
            for (w_r, dst) in ((wk_r, nkT), (wq_r, nqT)):
                for qc in range(4):
                    steps.append(lambda w_r=w_r, dst=dst, qc=qc:
                                 qk_step(h, w_r, dst, qc, "po", psOut,
                                         via_dma=True))
            return steps

        def make_epilogue(h, qcb, bs):
            # the very last block's epilogue is the kernel tail: split the
            # WHOLE pipeline (mask/fold/recip included) into two 256-wide
            # chains so the first y write fires as early as possible
            if h == NH - 1 and qcb == NQ - 1:
                chains = [_make_den_half(h, qcb, bs, off)
                          + _make_epilogue_part(h, qcb, bs, off, 256)
                          for off in (0, 256)]
                return [s for pair in zip(*chains) for s in pair]
            return (_make_den_steps(h, qcb, bs)
                    + _make_epilogue_part(h, qcb, bs, 0, 512))

        def _make_den_half(h, qcb, bs, off):
            if "rec_f" not in bs:
                bs["rec_f"] = small.tile([1, 512], f32, name=f"rec{h}{qcb}",
                                         tag="recf")
                bs["d0cI"] = small.tile([P, 512], f32r, name=f"dI{h}{qcb}",
                                        tag="d0ci")
                bs["d_row"] = psDen.tile([1, 512], f32, name=f"dr{h}{qcb}",
                                         tag="den")
            rec_f, d0cI, d_row = bs["rec_f"], bs["d0cI"], bs["d_row"]
            j0 = off // P
            steps = []

            def mask():
                d0c = bs["d0c"]
                in_id = (ident[:].rearrange("p (a w) -> p a w", a=1)
                         .broadcast_to((P, 2, P)))
                in_d = (d0c[:, 2 * j0:2 * j0 + 4]
                        .rearrange("p (j two) -> p j two", two=2)
                        [:, :, 0:1].broadcast_to((P, 2, P)))
                out_v = (d0cI[:, off:off + 256]
                         .rearrange("p (j w) -> p j w", w=P))
                # all-SBUF op: the second half can ride the idle Pool engine
                eng = nc.vector if off == 0 else nc.gpsimd
                eng.tensor_mul(out_v, in_id, in_d)
            steps.append(mask)

            def fold_row():
                nc.tensor.matmul(d_row[0:1, off:off + 256], ones_col[:],
                                 d0cI[:, off:off + 256],
                                 start=True, stop=True)
            steps.append(fold_row)

            def recip():
                nc.vector.reciprocal_approx_fast(rec_f[0:1, off:off + 256],
                                                 d_row[0:1, off:off + 256])
            steps.append(recip)
            return steps

        def _make_den_steps(h, qcb, bs):
            # flip d0c [128q-part, 8] into a [1,512] denominator ROW without
            # a transpose: d0cI[k,(j,r)] = ident[k,r] * d0c[k,2j], then a
            # single 512-col ones fold gives d_row[0, j*128+r] = d0c[r, 2j]
            rec_f = small.tile([1, 512], f32, name=f"rec{h}{qcb}", tag="recf")
            bs["rec_f"] = rec_f
            steps = []

            def mask():
                d0cI = small.tile([P, 512], f32r, name=f"dI{h}{qcb}",
                                  tag="d0ci")
                bs["d0cI"] = d0cI
                d0c = bs["d0c"]
                in_id = (ident[:].rearrange("p (a w) -> p a w", a=1)
                         .broadcast_to((P, 4, P)))
                in_d = (d0c[:].rearrange("p (j two) -> p j two", two=2)
                        [:, :, 0:1].broadcast_to((P, 4, P)))
                out_v = d0cI[:].rearrange("p (j w) -> p j w", w=P)
                nc.vector.tensor_mul(out_v, in_id, in_d)
            steps.append(mask)

            def fold_row():
                d_row = psDen.tile([1, 512], f32, name=f"dr{h}{qcb}",
                                   tag="den")
                bs["d_row"] = d_row
                nc.tensor.matmul(d_row[:], ones_col[:], bs["d0cI"][:],
                                 start=True, stop=True)
            steps.append(fold_row)

            def recip():
                nc.vector.reciprocal_approx_fast(rec_f[:], bs["d_row"][:])
            steps.append(recip)
            return steps

        def _make_epilogue_part(h, qcb, bs, off, w):
            q0 = qcb * 512 + off
            outTn, outPS, rec_f = bs["outTn"], bs["outPS"], bs["rec_f"]
            steps = []
            tail = h == NH - 1 and qcb == NQ - 1

            bcs = small.tile([P, w], f32, name=f"bcs{h}{qcb}{off}",
                             tag="bc_sb")

            def bcast():
                if tail:
                    # kernel tail: avoid the ~2.5us DMA round trip with a
                    # rank-1 broadcast matmul from the denominator row
                    bc = psStage.tile([P, w], f32, name=f"bc{h}{qcb}{off}",
                                      tag="stage")
                    nc.tensor.matmul(bc[:], ones_row_f[:],
                                     rec_f[0:1, off:off + w],
                                     start=True, stop=True)
                    # Act is idle at the tail: drain via activation-copy
                    nc.scalar.copy(bcs[:], bc[:])
                else:
                    # one stride-0 broadcast DMA; the trigger rides SP so
                    # Act's sequencer never blocks behind its sem wait
                    nc.sync.dma_start(bcs[:],
                                      rec_f[0:1, off:off + w]
                                      .rearrange("(a b) w -> a b w", b=1)
                                      .broadcast_to((1, P, w)))
            steps.append(bcast)

            def norm():
                nc.vector.tensor_mul(outTn[:, q0:q0 + w],
                                     outPS[:, off:off + w], bcs[:])
            steps.append(norm)

            def yacc():
                psy = psDen.tile([P, w], f32, name=f"psy{h}{qcb}{off}",
                                 tag="den")

                nc.tensor.matmul(psy[:], wo_r[:, h * P:(h + 1) * P],
                                 outTn[:, q0:q0 + w], start=True, stop=True)
                if h == 0:
                    nc.vector.tensor_copy(yT[:, q0:q0 + w], psy[:])
                else:
                    nc.vector.tensor_add(yT[:, q0:q0 + w],
                                         yT[:, q0:q0 + w], psy[:])
                if h == NH - 1:
                    nc.sync.dma_start(y[:, q0:q0 + w], yT[:, q0:q0 + w])
            steps.append(yacc)
            return steps

        # consumer lag: LAG mid-stream, tapering for the last block so the
        # tail drain after the final exp is short
        cons = 0
        total_pairs = NB * NP

        def lag_of(ci):
            return LAG if ci < (NB - 1) * NP else 3

        for j in range(NB * NP + LAG):
            if j < NB * NP:
                b, p = divmod(j, NP)
                h, qcb = blocks[b]
                if p == 0:
                    qT, kT = heads_qk[h]
                    bs = bstate[b] = {
                        "qT": qT, "kT": kT,
                        "outTn": (bstate[b - 1]["outTn"]
                                  if qcb != 0 else
                                  outp.tile([P, S], f32r, name=f"outTn{h}",
                                            tag="outTn")),
                        "outPS": psOut.tile([P, 512], f32, name=f"oPS{h}{qcb}",
                                            tag="po"),
                        "exs": [None] * NP,
                    }
                    if qcb == NQ - 3 and h + 1 < NH:
                        bg_pre.extend(make_prefetch(h + 1))
                else:
                    bs = bstate[b]
                q0 = qcb * 512
                st = psStage.tile([P, 1024], f32, name=f"st{h}{qcb}{p}",
                                  tag="stage")
                for half in range(2):
                    kc = 2 * p + half
                    nc.tensor.matmul(st[:, half * 512:(half + 1) * 512],
                                     bs["kT"][:, kc * P:(kc + 1) * P],
                                     bs["qT"][:, q0:q0 + 512],
                                     start=True, stop=True)
                ex = expp.tile([P, 1024], f32r, name=f"ex{h}{qcb}{p}",
                               tag="exp")
                nc.scalar.activation(ex[:], st[:], Exp)
                bs["exs"][p] = ex
            while cons < total_pairs and cons <= j - lag_of(cons):
                jj = cons
                cons += 1
                b2, p2 = divmod(jj, NP)
                h2, qcb2 = blocks[b2]
                bs2 = bstate[b2]
                exp_pair = bs2["exs"][p2]
                for half in range(2):
                    k2 = 2 * p2 + half
                    exh = exp_pair[:, half * 512:(half + 1) * 512]
                    vh = v_sb[:, k2 * 512 + h2 * P:k2 * 512 + (h2 + 1) * P]
                    nc.tensor.matmul(bs2["outPS"][:], vh, exh,
                                     start=(k2 == 0), stop=(k2 == SC - 1))
                # denominator: 8 tiny fold matmuls per pair with the EXP
                # tile as the STATIONARY operand and a ones column moving —
                # out free size is 1, so each costs ~2ns of PE (the cost
                # model charges matmuls by output free size; LdWeights is
                # free).  d0ps[q%128, q//128] accumulates sum_k exp over the
                # block's 16 k-chunks in 4 psum accumulation groups.
                for half in range(2):
                    for jq in range(4):
                        c0 = half * 512 + jq * P
                        nc.tensor.matmul(dall[:, 2 * jq:2 * jq + 2],
                                         exp_pair[:, c0:c0 + P],
                                         ones_c2[:],
                                         start=(p2 == 0 and half == 0
                                                and jq == 0),
                                         stop=(p2 == NP - 1 and half == 1
                                               and jq == 3))
                if p2 == NP - 1:
                    # drain the fold accumulators NOW (inline) so the next
                    # block's start=True folds can't clobber them
                    d0c = small.tile([P, 8], f32, name=f"d0c{h2}{qcb2}",
                                     tag="d0c")
                    nc.vector.tensor_copy(d0c[:], dall[:, 0:8])
                    bs2["d0c"] = d0c
                    bg_epi.extend(make_epilogue(h2, qcb2, bs2))
                    bstate.pop(b2 - 1, None)
            # drip-feed background work, epilogues first (release PSUM
            # slots).  Nothing drips on block-start iterations (p == 0) so
            # the next block's scores aren't queued behind background
            # matmuls on the in-order PE; startup drips at 1/iter for the
            # first irons so the first exps aren't sem-gated behind v-steps
            n_su = 1 if j == 0 else 2
            for _ in range(n_su):
                if startup_bg:
                    startup_bg.pop(0)()
            if bg_epi:
                bg_epi.pop(0)()
            if bg_pre and j % 3 == 2 and j < (NB - 1) * NP:
                bg_pre.pop(0)()
            elif bg_epi:
                bg_epi.pop(0)()
        while bg_epi or bg_pre:
            (bg_epi or bg_pre).pop(0)()

    nc.compile()
    return nc


def _get_nc():
    if "nc" not in _CACHE:
        _CACHE["nc"] = _build()
    return _CACHE["nc"]


def kernel(x, Wq, Wk, Wv, Wo, bo):
    global LAST_RESULTS
    from concourse.bass_utils import run_bass_kernel_spmd

    x = np.asarray(x, dtype=np.float32)
    Wq = np.asarray(Wq, dtype=np.float32)
    Wk = np.asarray(Wk, dtype=np.float32)
    Wv = np.asarray(Wv, dtype=np.float32)
    Wo = np.asarray(Wo, dtype=np.float32)
    bo = np.asarray(bo, dtype=np.float32)

    nc = _get_nc()
    qk_scale = np.float32(P ** -0.5)
    in_maps = []
    for core in range(N_CORES):
        b, hg = core // 2, core % 2
        cols = slice(hg * NH * P, (hg + 1) * NH * P)
        in_maps.append({
            "xt": _tf32_round(x[b].T),
            "wq": _tf32_round(Wq[:, cols] * qk_scale),
            "wk": _tf32_round(Wk[:, cols]),
            "wv": _tf32_round(Wv[:, cols]),
            # [d, h*128+c] so wo_r[:, h*128:(h+1)*128] is wo_h as [d, c]
            "wo": _tf32_round(Wo[cols, :].reshape(NH, P, P)
                              .transpose(1, 0, 2).reshape(P, NH * P)),
        })
    trace = bool(int(os.environ.get("KERNEL_TRACE", "0")))
    res = run_bass_kernel_spmd(nc, in_maps, core_ids=list(range(N_CORES)),
                               trace=trace)
    LAST_RESULTS = res
    parts = [np.ascontiguousarray(r["y"].T) for r in res.results]
    out = np.stack([parts[2 * b] + parts[2 * b + 1] + bo[None, :]
                    for b in range(4)])
    return out.astype(np.float32)

